# revision 1
# baseline (speedup 1.0000x reference)
"""Trainium2 Bass kernel for nn_ContinousNormalizingFlowRHS.

Computes, for z in R^{B x Z} and scalar time t:
  h0 = tanh(W1*t + B1); h1 = tanh(einsum('knm,km->kn', W2, h0) + B2)
  w_in  = (W3_win  @ h1[0] + b3_win ).reshape(F, Z)
  w_out = (W3_wout @ h1[1] + b3_wout).reshape(F, Z)
  b     =  W3_b    @ h1[2] + b3_b
  gate  = sigmoid(W3_gate @ h1[3] + b3_gate)
  h = tanh(z @ w_in.T + b); dz = (h*gate) @ w_out / F
  trace = ((1-h^2)*gate) @ (sum(w_in*w_out,1)) / F
  out = concat([dz, -trace[:,None]], -1)

Strategy (8 NeuronCores, single SPMD launch):
  The dominant cost is streaming W3_win/W3_wout (268 MB each) for the
  matvecs, so those are sharded row-wise across the 8 cores (F-sharding).
  Each core's matvec work is further split between the PE (transposed
  bf16 slices as stationary weights, h1 column as the moving operand)
  and the DVE (natural-layout slices, multiply by a partition-broadcast
  h1 then reduce along the free axis), so neither engine is the
  bottleneck and the HBM stream rate binds.  Each core then runs the
  batch matmuls for the FULL batch against its local f-slice, producing
  partial dz/trace sums.  Two pipelined ReduceScatter(add) ops complete
  the sum over F and hand each core its own batch shard of the output.
"""

import sys
import types
import numpy as np
import ml_dtypes

BF = ml_dtypes.bfloat16

# problem sizes (hardcoded per contract)
Z = 128
N = 256
F = 2048
B = 8192
N_CORES = 8

PE_COLS = 128       # per matrix: f-columns computed on the PE (rest on DVE)
CHUNK_R = 4096      # W3 rows per streamed PE chunk ([128, 4096] bf16 tiles)
DVE_CC = 16         # f-columns per DVE chunk (2048 rows)
BC = 512            # batch columns per stage-B chunk (one PSUM bank)


def _ensure_ntff_hook():
    """run_bass_kernel_spmd(trace=True) under axon needs antenv.axon_hooks."""
    if 'antenv.axon_hooks' in sys.modules:
        return
    try:
        from trn_agent_boot.trn_boot import _ntff_profile_via_ctypes
        hook = _ntff_profile_via_ctypes('/opt/axon/libaxon_pjrt.so')
    except Exception:
        hook = None
    try:
        import antenv
    except Exception:
        return
    mod = types.ModuleType('antenv.axon_hooks')
    mod.get_axon_ntff_profile_hook = lambda: hook
    mod.set_axon_ntff_profile_hook = lambda h: None
    sys.modules['antenv.axon_hooks'] = mod
    antenv.axon_hooks = mod


def build_module(n_cores=N_CORES, b=B, f=F, pe_cols=PE_COLS, chunk_r=CHUNK_R,
                 bc=BC, debug=False):
    """Build the Bass module (SPMD program, one per core)."""
    import concourse.tile as tile
    from concourse import bacc, mybir

    F32 = mybir.dt.float32
    BF16 = mybir.dt.bfloat16
    ADD = mybir.AluOpType.add

    fl = f // n_cores            # local f count
    nfb = fl // 128              # local f blocks of 128
    rows_pe = pe_cols * 128      # rows of W3 handled by the PE
    dve_cols = fl - pe_cols
    rows_dve = dve_cols * 128
    n_pe_chunks = rows_pe // chunk_r
    rpc = chunk_r // 128         # w columns produced per PE chunk
    dcc = DVE_CC                 # f-columns per DVE chunk
    n_dve_chunks = dve_cols // dcc
    bl = b // n_cores            # output batch shard
    hw = bl // 2                 # reduce-scatter half width
    assert rows_pe % chunk_r == 0 and dve_cols % dcc == 0
    assert hw % bc == 0

    nc = bacc.Bacc("TRN2", target_bir_lowering=False, debug=debug,
                   num_devices=n_cores)

    def inp(name, shape, dt):
        return nc.dram_tensor(name, shape, dt, kind="ExternalInput").ap()

    t_ap = inp("t", [1, 1], F32)
    w1_ap = inp("w1c", [128, 8], F32)
    b1_ap = inp("b1c", [128, 8], F32)
    b2_ap = inp("b2c", [128, 8], F32)
    w2t_ap = inp("w2tc", [128, 2048], BF16)
    w3winT_ap = inp("w3winT_sl", [N, rows_pe], BF16)
    w3woutT_ap = inp("w3woutT_sl", [N, rows_pe], BF16)
    w3winN_ap = inp("w3winN_sl", [rows_dve // (dcc * 128) * 128, dcc * N], BF16)
    w3woutN_ap = inp("w3woutN_sl", [rows_dve // (dcc * 128) * 128, dcc * N], BF16)
    b3win_ap = inp("b3win_c", [128, fl], F32)
    b3wout_ap = inp("b3wout_c", [128, fl], F32)
    w3bT_ap = inp("w3bT_sl", [N, fl], BF16)
    w3gateT_ap = inp("w3gateT_sl", [N, fl], BF16)
    b3b_ap = inp("b3b_c", [128, nfb], F32)
    b3gate_ap = inp("b3gate_c", [128, nfb], F32)
    zt_ap = inp("ztb", [128, b], BF16)
    eye_ap = inp("eyeb", [128, 128], BF16)
    out_ap = nc.dram_tensor("out", [Z + 1, bl], F32, kind="ExternalOutput").ap()

    with tile.TileContext(nc) as tc:
        with tc.tile_pool(name="persist", bufs=1) as pp, \
             tc.tile_pool(name="stream", bufs=4) as sp, \
             tc.tile_pool(name="work", bufs=3) as wp, \
             tc.tile_pool(name="ps_h", bufs=2, space="PSUM") as ps_h, \
             tc.tile_pool(name="ps_dz", bufs=2, space="PSUM") as ps_dz, \
             tc.tile_pool(name="ps_t2", bufs=2, space="PSUM") as ps_t2, \
             tc.tile_pool(name="ps_prep", bufs=2, space="PSUM") as ps_prep, \
             tc.tile_pool(name="dram", bufs=1, space="DRAM") as dp:

            # ---- parameter nets (tiny) ----------------------------------
            t_bc = pp.tile([128, 1], F32, tag="tbc")
            nc.gpsimd.dma_start(t_bc[:], t_ap.broadcast_to([128, 1]))
            w1_sb = pp.tile([128, 8], F32, tag="w1")
            b1_sb = pp.tile([128, 8], F32, tag="b1")
            b2_sb = pp.tile([128, 8], F32, tag="b2")
            w2t_sb = pp.tile([128, 2048], BF16, tag="w2t")
            nc.gpsimd.dma_start(w1_sb[:], w1_ap[:])
            nc.gpsimd.dma_start(b1_sb[:], b1_ap[:])
            nc.gpsimd.dma_start(b2_sb[:], b2_ap[:])
            nc.gpsimd.dma_start(w2t_sb[:], w2t_ap[:])

            h0pre = pp.tile([128, 8], F32, tag="h0pre")
            nc.vector.tensor_scalar_mul(h0pre[:], w1_sb[:], t_bc[:, 0:1])
            nc.vector.tensor_add(h0pre[:], h0pre[:], b1_sb[:])
            h0_sb = pp.tile([128, 8], BF16, tag="h0")
            nc.scalar.activation(h0_sb[:], h0pre[:],
                                 mybir.ActivationFunctionType.Tanh)

            ps_h1 = ps_prep.tile([128, 8], F32, tag="prep")
            for k4 in range(4):
                for nb in range(2):
                    c = k4 * 2 + nb
                    for mb in range(2):
                        lhs = w2t_sb[:, k4 * 512 + mb * 256 + nb * 128:
                                     k4 * 512 + mb * 256 + nb * 128 + 128]
                        nc.tensor.matmul(ps_h1[:, c:c + 1], lhs,
                                         h0_sb[:, k4 * 2 + mb:k4 * 2 + mb + 1],
                                         start=(mb == 0), stop=(mb == 1))
            h1pre = pp.tile([128, 8], F32, tag="h1pre")
            h1_sb = pp.tile([128, 8], BF16, tag="h1")
            nc.vector.tensor_add(h1pre[:], ps_h1[:], b2_sb[:])
            nc.scalar.activation(h1_sb[:], h1pre[:],
                                 mybir.ActivationFunctionType.Tanh)
            # h1 -> DRAM in (net, n) order, then broadcast-load nets 0/1
            # replicated across partitions AND repeated dcc times along the
            # free dim (so the DVE multiply runs chunk-granular).
            h1_dram = dp.tile([8, 128], BF16, tag="h1d")
            nc.gpsimd.dma_start(h1_dram.rearrange("c n -> n c"), h1_sb[:])
            h1b = []
            for k4 in range(2):
                hb = pp.tile([128, dcc * N], BF16, tag=f"h1b{k4}")
                src = h1_dram.rearrange("c n -> (c n)")[k4 * N:(k4 + 1) * N]
                src = src.unsqueeze(0).unsqueeze(0)
                nc.gpsimd.dma_start(hb[:], src.broadcast_to([128, dcc, N]))
                h1b.append(hb)

            # ---- phase 1: sharded matvecs, split across PE and DVE ------
            b3win_sb = pp.tile([128, fl], F32, tag="b3win")
            b3wout_sb = pp.tile([128, fl], F32, tag="b3wout")
            nc.scalar.dma_start(b3win_sb[:], b3win_ap[:])
            nc.scalar.dma_start(b3wout_sb[:], b3wout_ap[:])

            w_inT_bf = pp.tile([128, fl], BF16, tag="winT")
            w_outT_bf = pp.tile([128, fl], BF16, tag="woutT")

            # PE part: columns [0, pe_cols) of each matrix
            for w3T_ap, bias_sb, dst, net in ((w3winT_ap, b3win_sb, w_inT_bf, 0),
                                              (w3woutT_ap, b3wout_sb, w_outT_bf, 1)):
                for c in range(n_pe_chunks):
                    tiles = []
                    for nb in range(2):
                        w3t = sp.tile([128, chunk_r], BF16, tag="w3chunk")
                        nc.sync.dma_start(
                            w3t[:], w3T_ap[nb * 128:(nb + 1) * 128,
                                           c * chunk_r:(c + 1) * chunk_r])
                        tiles.append(w3t)
                    pw = ps_prep.tile([128, rpc], F32, tag="prep")
                    for a in range(rpc):
                        for nb in range(2):
                            nc.tensor.matmul(
                                pw[:, a:a + 1],
                                tiles[nb][:, a * 128:(a + 1) * 128],
                                h1_sb[:, net * 2 + nb:net * 2 + nb + 1],
                                start=(nb == 0), stop=(nb == 1))
                    nc.vector.tensor_add(dst[:, c * rpc:(c + 1) * rpc], pw[:],
                                         bias_sb[:, c * rpc:(c + 1) * rpc])

            # DVE part: columns [pe_cols, fl) of each matrix, one chunk-wide
            # multiply + one 3-D reduce per dcc columns.
            for w3N_ap, bias_sb, dst, net in ((w3winN_ap, b3win_sb, w_inT_bf, 0),
                                              (w3woutN_ap, b3wout_sb, w_outT_bf, 1)):
                acc = pp.tile([128, max(dve_cols, 1)], F32, tag=f"dacc{net}")
                for c in range(n_dve_chunks):
                    w3n = sp.tile([128, dcc * N], BF16, tag="w3nat")
                    nc.scalar.dma_start(w3n[:],
                                        w3N_ap[c * 128:(c + 1) * 128, :])
                    prod = wp.tile([128, dcc * N], BF16, tag="prod")
                    nc.vector.tensor_mul(prod[:], w3n[:], h1b[net][:])
                    nc.vector.tensor_reduce(
                        acc[:, c * dcc:(c + 1) * dcc],
                        prod.rearrange("p (a n) -> p a n", a=dcc),
                        mybir.AxisListType.X, ADD)
                if dve_cols:
                    nc.vector.tensor_add(dst[:, pe_cols:fl], acc[:, 0:dve_cols],
                                         bias_sb[:, pe_cols:fl])

            # heads: b and gate (psum [f, fb] columns)
            b3b_sb = pp.tile([128, nfb], F32, tag="b3b")
            b3gate_sb = pp.tile([128, nfb], F32, tag="b3gate")
            nc.gpsimd.dma_start(b3b_sb[:], b3b_ap[:])
            nc.gpsimd.dma_start(b3gate_sb[:], b3gate_ap[:])
            b_sb = pp.tile([128, nfb], F32, tag="bh")
            gate_sb = pp.tile([128, nfb], F32, tag="gate")
            gpre = pp.tile([128, nfb], F32, tag="gpre")
            for w3hT_ap, bias_sb, dst, net in ((w3bT_ap, b3b_sb, b_sb, 2),
                                               (w3gateT_ap, b3gate_sb, gpre, 3)):
                w3ht = sp.tile([128, 2 * fl], BF16, tag="w3head")
                nc.scalar.dma_start(
                    w3ht[:], w3hT_ap.rearrange("(nb p) fl -> p nb fl", p=128))
                phd = ps_prep.tile([128, nfb], F32, tag="prep")
                for a in range(nfb):
                    for nb in range(2):
                        nc.tensor.matmul(
                            phd[:, a:a + 1],
                            w3ht[:, nb * fl + a * 128:nb * fl + (a + 1) * 128],
                            h1_sb[:, net * 2 + nb:net * 2 + nb + 1],
                            start=(nb == 0), stop=(nb == 1))
                nc.vector.tensor_add(dst[:], phd[:], bias_sb[:])
            nc.scalar.activation(gate_sb[:], gpre[:],
                                 mybir.ActivationFunctionType.Sigmoid)

            # ---- stage-B constants --------------------------------------
            zt_sb = pp.tile([128, b], BF16, tag="zt")
            nc.scalar.dma_start(zt_sb[:], zt_ap[:])
            eye_sb = pp.tile([128, 128], BF16, tag="eye")
            nc.gpsimd.dma_start(eye_sb[:], eye_ap[:])

            # transpose w_in/w_out to [f, z]; fold gate into w_out
            w_outg = pp.tile([128, nfb * 128], BF16, tag="woutg")
            w_in_fz = pp.tile([128, nfb * 128], BF16, tag="winfz")
            sg = pp.tile([128, nfb], F32, tag="sg")
            for fb in range(nfb):
                ptr = ps_prep.tile([128, 128], BF16, tag="prep")
                nc.tensor.transpose(ptr[:], w_outT_bf[:, fb * 128:(fb + 1) * 128],
                                    eye_sb[:])
                nc.vector.tensor_scalar_mul(w_outg[:, fb * 128:(fb + 1) * 128],
                                            ptr[:], gate_sb[:, fb:fb + 1])
                pti = ps_prep.tile([128, 128], BF16, tag="prep")
                nc.tensor.transpose(pti[:], w_inT_bf[:, fb * 128:(fb + 1) * 128],
                                    eye_sb[:])
                nc.vector.tensor_copy(w_in_fz[:, fb * 128:(fb + 1) * 128], pti[:])
                # s' = sum_z w_in[f,z] * w_out[f,z] * gate[f]
                prod = wp.tile([128, 128], F32, tag="sprod")
                nc.vector.tensor_mul(prod[:], w_in_fz[:, fb * 128:(fb + 1) * 128],
                                     w_outg[:, fb * 128:(fb + 1) * 128])
                nc.vector.tensor_reduce(sg[:, fb:fb + 1], prod[:],
                                        mybir.AxisListType.X, ADD)
            sg_bf = pp.tile([128, nfb], BF16, tag="sgbf")
            nc.vector.tensor_copy(sg_bf[:], sg[:])
            # cneg = -sum_f s' / F
            csum = pp.tile([1, 1], F32, tag="csum")
            nc.gpsimd.tensor_reduce(csum[:], sg[:], mybir.AxisListType.XYZWC, ADD)
            cneg = pp.tile([1, 1], F32, tag="cneg")
            nc.scalar.mul(cneg[:], csum[:], -1.0 / f)

            # ---- stage B: batch matmuls over local f slice --------------
            # half h of every core's [Z+1, bl] output reduces in its own
            # ReduceScatter so the first one overlaps remaining compute.
            cc_in = [dp.tile([n_cores, Z, hw], BF16, tag=f"ccin{h}",
                             name=f"ccin{h}") for h in range(2)]
            cc_out = [dp.tile([Z, hw], BF16, tag=f"ccout{h}",
                              name=f"ccout{h}") for h in range(2)]
            cc_tr_in = dp.tile([n_cores, bl], F32, tag="cctri", name="cctri")
            cc_tr_out = dp.tile([1, bl], F32, tag="cctro", name="cctro")
            for half in range(2):
                for kk in range(n_cores):
                    for j in range(hw // bc):
                        g0 = kk * bl + half * hw + j * bc
                        pdz = ps_dz.tile([128, bc], F32, tag="pdz")
                        pt2 = ps_t2.tile([1, bc], F32, tag="pt2")
                        for fb in range(nfb):
                            ph = ps_h.tile([128, bc], F32, tag="ph")
                            nc.tensor.matmul(ph[:],
                                             w_inT_bf[:, fb * 128:(fb + 1) * 128],
                                             zt_sb[:, g0:g0 + bc],
                                             start=True, stop=True)
                            h_bf = wp.tile([128, bc], BF16, tag="hbf")
                            nc.scalar.activation(
                                h_bf[:], ph[:],
                                mybir.ActivationFunctionType.Tanh,
                                bias=b_sb[:, fb:fb + 1])
                            h2_bf = wp.tile([128, bc], BF16, tag="h2bf")
                            nc.vector.tensor_mul(h2_bf[:], h_bf[:], h_bf[:])
                            nc.tensor.matmul(pdz[:],
                                             w_outg[:, fb * 128:(fb + 1) * 128],
                                             h_bf[:],
                                             start=(fb == 0), stop=(fb == nfb - 1))
                            nc.tensor.matmul(pt2[:], sg_bf[:, fb:fb + 1], h2_bf[:],
                                             start=(fb == 0), stop=(fb == nfb - 1))
                        dz_sb = wp.tile([128, bc], BF16, tag="dzsb")
                        nc.scalar.mul(dz_sb[:], pdz[:], 1.0 / f)
                        tr_sb = wp.tile([1, bc], F32, tag="trsb")
                        nc.scalar.activation(
                            tr_sb[:], pt2[:],
                            mybir.ActivationFunctionType.Identity,
                            bias=cneg[0:1, 0:1], scale=1.0 / f)
                        off = j * bc
                        nc.sync.dma_start(cc_in[half][kk, :, off:off + bc],
                                          dz_sb[:])
                        nc.sync.dma_start(
                            cc_tr_in[kk, half * hw + off:half * hw + off + bc]
                            .unsqueeze(0), tr_sb[:])
                nc.gpsimd.collective_compute(
                    "ReduceScatter", ADD,
                    replica_groups=[list(range(n_cores))],
                    ins=[cc_in[half].opt()], outs=[cc_out[half].opt()])
                nc.gpsimd.dma_start(out_ap[0:Z, half * hw:(half + 1) * hw],
                                    cc_out[half][:])
            nc.gpsimd.collective_compute(
                "ReduceScatter", ADD,
                replica_groups=[list(range(n_cores))],
                ins=[cc_tr_in.opt()], outs=[cc_tr_out.opt()])
            nc.gpsimd.dma_start(out_ap[Z:Z + 1, :], cc_tr_out[:])

    nc.compile()
    return nc


def host_prep(t, z_and_logpz, W1, B1, W2, B2, W3_win, b3_win,
              W3_wout, b3_wout, W3_b, b3_b, W3_gate, b3_gate,
              n_cores=N_CORES, b=B, f=F, pe_cols=PE_COLS):
    """Shard + lay out the numpy inputs into per-core in_maps."""
    fl = f // n_cores
    nfb = fl // 128
    rows = fl * Z
    rows_pe = pe_cols * 128

    dcc = DVE_CC

    def pack_nat(x):  # [rows_dve, N] -> [nch*128, dcc*N], partition-contiguous
        nch = x.shape[0] // (dcc * 128)
        return np.ascontiguousarray(
            x.reshape(nch, dcc, 128, N).transpose(0, 2, 1, 3)
            .reshape(nch * 128, dcc * N))

    def col8(x):  # [4, 256] -> [128, 8] with col = k*2 + nb
        return np.ascontiguousarray(
            np.asarray(x, np.float32).reshape(4, 2, 128).transpose(2, 0, 1)
            .reshape(128, 8))

    t_in = np.asarray(t, np.float32).reshape(1, 1)
    w1c = col8(np.asarray(W1, np.float32)[:, :, 0])
    b1c = col8(B1)
    b2c = col8(B2)
    # lhsT tile for h1 net: [m128, (k4, mb, n)] = W2[k4, n, mb*128+m128]
    w2tc = np.ascontiguousarray(
        np.asarray(W2, np.float32).transpose(0, 2, 1)        # [k, m, n]
        .reshape(4, 2, 128, 256).transpose(2, 0, 1, 3).reshape(128, 2048)).astype(BF)
    w3win_bf = np.asarray(W3_win, np.float32).astype(BF)
    w3wout_bf = np.asarray(W3_wout, np.float32).astype(BF)
    w3b_bf = np.asarray(W3_b, np.float32).astype(BF)
    w3gate_bf = np.asarray(W3_gate, np.float32).astype(BF)
    b3win = np.asarray(b3_win, np.float32)
    b3wout = np.asarray(b3_wout, np.float32)
    b3b = np.asarray(b3_b, np.float32)
    b3gate = np.asarray(b3_gate, np.float32)
    z = np.asarray(z_and_logpz, np.float32)[:, :Z]
    ztb = np.ascontiguousarray(z.T).astype(BF)
    eye = np.eye(128, dtype=np.float32).astype(BF)

    in_maps = []
    for k in range(n_cores):
        r0 = k * rows
        f0 = k * fl
        in_maps.append({
            "t": t_in, "w1c": w1c, "b1c": b1c, "b2c": b2c, "w2tc": w2tc,
            "w3winT_sl": np.ascontiguousarray(w3win_bf[r0:r0 + rows_pe].T),
            "w3woutT_sl": np.ascontiguousarray(w3wout_bf[r0:r0 + rows_pe].T),
            "w3winN_sl": pack_nat(w3win_bf[r0 + rows_pe:r0 + rows]),
            "w3woutN_sl": pack_nat(w3wout_bf[r0 + rows_pe:r0 + rows]),
            "b3win_c": np.ascontiguousarray(
                b3win[r0:r0 + rows].reshape(fl, 128).T),
            "b3wout_c": np.ascontiguousarray(
                b3wout[r0:r0 + rows].reshape(fl, 128).T),
            "w3bT_sl": np.ascontiguousarray(w3b_bf[f0:f0 + fl].T),
            "w3gateT_sl": np.ascontiguousarray(w3gate_bf[f0:f0 + fl].T),
            "b3b_c": np.ascontiguousarray(b3b[f0:f0 + fl].reshape(nfb, 128).T),
            "b3gate_c": np.ascontiguousarray(
                b3gate[f0:f0 + fl].reshape(nfb, 128).T),
            "ztb": ztb, "eyeb": eye,
        })
    return in_maps


_NC_CACHE = {}


def kernel(**inputs) -> np.ndarray:
    _ensure_ntff_hook()
    from concourse import bass_utils

    key = "full"
    if key not in _NC_CACHE:
        _NC_CACHE[key] = build_module()
    nc = _NC_CACHE[key]

    in_maps = host_prep(**inputs)
    res = bass_utils.run_bass_kernel_spmd(nc, in_maps, list(range(N_CORES)))
    bl = B // N_CORES
    out = np.empty((B, Z + 1), np.float32)
    for k in range(N_CORES):
        out[k * bl:(k + 1) * bl, :] = res.results[k]["out"].T
    return out



# revision 14
# speedup vs baseline: 1.7892x; 1.7892x over previous
"""Trainium2 Bass kernel for nn_ContinousNormalizingFlowRHS.

Computes, for z in R^{B x Z} and scalar time t:
  h0 = tanh(W1*t + B1); h1 = tanh(einsum('knm,km->kn', W2, h0) + B2)
  w_in  = (W3_win  @ h1[0] + b3_win ).reshape(F, Z)
  w_out = (W3_wout @ h1[1] + b3_wout).reshape(F, Z)
  b     =  W3_b    @ h1[2] + b3_b
  gate  = sigmoid(W3_gate @ h1[3] + b3_gate)
  h = tanh(z @ w_in.T + b); dz = (h*gate) @ w_out / F
  trace = ((1-h^2)*gate) @ (sum(w_in*w_out,1)) / F
  out = concat([dz, -trace[:,None]], -1)

Strategy (8 NeuronCores, single SPMD launch):
  Phase A (F-sharded): each core streams its 1/8 of W3_win/W3_wout
  (bf16, [N, rows] transposed layout) on the two HWDGE queues and runs
  the matvec entirely on the PE as a stream of FWL stationary loads
  with a 1-column moving h1.  The psum naturally lands in w_inT
  ([z, f]) layout.  w_out is transposed on-chip, gate/bias folded, and
  the per-f trace weights sg are reduced locally.
  Handoff: two chunked AllGathers (one per local f-half) move the tiny
  per-core (w_inT, w_outg_fz, sg, b) slices to every core, overlapped
  with the second half of the stream.
  Phase B (B-sharded): each core computes only its own B/8 batch rows
  against the full F, accumulating dz/trace in PSUM across all 16
  f-blocks, then writes its output shard directly (no ReduceScatter).
"""

import sys
import types
import numpy as np
import ml_dtypes

BF = ml_dtypes.bfloat16

# problem sizes (hardcoded per contract)
Z = 128
N = 256
F = 2048
B = 8192
N_CORES = 8

FL = F // N_CORES          # local f per core (256)
NQ = 2                     # AllGather chunks (f-halves of 128)
FQ = FL // NQ              # f per chunk (128)
RQ = FQ * Z                # W3 rows per chunk (16384)
CHUNK_R = 4096             # rows per streamed tile ([128, 4096] bf16)
BL = B // N_CORES          # batch shard (1024)
BC = 512                   # batch columns per stage-B psum chunk
CCW = 2 * FQ + 2           # cc payload cols: w_inT | w_outg_fz | sg | b


def _ensure_ntff_hook():
    """run_bass_kernel_spmd(trace=True) under axon needs antenv.axon_hooks."""
    if 'antenv.axon_hooks' in sys.modules:
        return
    try:
        from trn_agent_boot.trn_boot import _ntff_profile_via_ctypes
        hook = _ntff_profile_via_ctypes('/opt/axon/libaxon_pjrt.so')
    except Exception:
        hook = None
    try:
        import antenv
    except Exception:
        return
    mod = types.ModuleType('antenv.axon_hooks')
    mod.get_axon_ntff_profile_hook = lambda: hook
    mod.set_axon_ntff_profile_hook = lambda h: None
    sys.modules['antenv.axon_hooks'] = mod
    antenv.axon_hooks = mod


def build_module(n_cores=N_CORES, debug=False):
    import concourse.tile as tile
    from concourse import bacc, mybir

    F32 = mybir.dt.float32
    BF16 = mybir.dt.bfloat16
    ADD = mybir.AluOpType.add

    nrc = RQ // CHUNK_R          # streamed tiles per (chunk, matrix, nb) (4)
    cpr = CHUNK_R // Z           # psum cols per streamed tile (32)

    nc = bacc.Bacc("TRN2", target_bir_lowering=False, debug=debug,
                   num_devices=n_cores)

    def inp(name, shape, dt):
        return nc.dram_tensor(name, shape, dt, kind="ExternalInput").ap()

    t_ap = inp("t", [1, 1], F32)
    w1_ap = inp("w1c", [128, 8], F32)
    b1_ap = inp("b1c", [128, 8], F32)
    b2_ap = inp("b2c", [128, 8], F32)
    w2t_ap = inp("w2tc", [128, 2048], BF16)
    w3T_aps = [[inp(f"w3{m}T_c{q}", [N, RQ], BF16) for q in range(NQ)]
               for m in ("win", "wout")]
    b3winT_ap = inp("b3winT_c", [128, FL], F32)
    b3woutT_ap = inp("b3woutT_c", [128, FL], F32)
    w3bT_ap = inp("w3bT_c", [N, FL], BF16)
    w3gateT_ap = inp("w3gateT_c", [N, FL], BF16)
    b3b_ap = inp("b3b_c", [128, NQ], F32)
    b3gate_ap = inp("b3gate_c", [128, NQ], F32)
    zt_ap = inp("ztk", [128, BL], BF16)
    eye_ap = inp("eyeb", [128, 128], BF16)
    out_ap = nc.dram_tensor("out", [Z + 1, BL], F32,
                            kind="ExternalOutput").ap()

    with tile.TileContext(nc) as tc:
        with tc.tile_pool(name="persist", bufs=1) as pp, \
             tc.tile_pool(name="stream", bufs=4) as sp, \
             tc.tile_pool(name="work", bufs=3) as wp, \
             tc.tile_pool(name="ccsb", bufs=2) as cp, \
             tc.tile_pool(name="ps_misc", bufs=1, space="PSUM") as ps_misc, \
             tc.tile_pool(name="ps_mv", bufs=1, space="PSUM") as ps_mv, \
             tc.tile_pool(name="ps_h", bufs=2, space="PSUM") as ps_h, \
             tc.tile_pool(name="ps_dz", bufs=1, space="PSUM") as ps_dz, \
             tc.tile_pool(name="ps_tr", bufs=1, space="PSUM") as ps_tr, \
             tc.tile_pool(name="dram", bufs=1, space="DRAM") as dp:

            # ---- small loads (SWDGE queue; issue before the big stream) --
            t_bc = pp.tile([128, 1], F32, tag="tbc")
            nc.gpsimd.dma_start(t_bc[:], t_ap.broadcast_to([128, 1]))
            w1_sb = pp.tile([128, 8], F32, tag="w1")
            b1_sb = pp.tile([128, 8], F32, tag="b1")
            b2_sb = pp.tile([128, 8], F32, tag="b2")
            w2t_sb = pp.tile([128, 2048], BF16, tag="w2t")
            nc.gpsimd.dma_start(w1_sb[:], w1_ap[:])
            nc.gpsimd.dma_start(b1_sb[:], b1_ap[:])
            nc.gpsimd.dma_start(b2_sb[:], b2_ap[:])
            nc.gpsimd.dma_start(w2t_sb[:], w2t_ap[:])
            zt_sb = pp.tile([128, BL], BF16, tag="zt")
            nc.gpsimd.dma_start(zt_sb[:], zt_ap[:])
            eye_sb = pp.tile([128, 128], BF16, tag="eye")
            nc.gpsimd.dma_start(eye_sb[:], eye_ap[:])
            b3winT_sb = pp.tile([128, FL], F32, tag="b3winT")
            b3woutT_sb = pp.tile([128, FL], F32, tag="b3woutT")
            nc.gpsimd.dma_start(b3winT_sb[:], b3winT_ap[:])
            nc.gpsimd.dma_start(b3woutT_sb[:], b3woutT_ap[:])
            w3h_sb = []
            for m, ap in (("b", w3bT_ap), ("gate", w3gateT_ap)):
                hb = pp.tile([128, 2 * FL], BF16, tag=f"w3{m}T")
                nc.gpsimd.dma_start(
                    hb[:], ap.rearrange("(nb p) fl -> p nb fl", p=128))
                w3h_sb.append(hb)
            b3b_sb = pp.tile([128, NQ], F32, tag="b3b")
            b3gate_sb = pp.tile([128, NQ], F32, tag="b3gate")
            nc.gpsimd.dma_start(b3b_sb[:], b3b_ap[:])
            nc.gpsimd.dma_start(b3gate_sb[:], b3gate_ap[:])

            # ---- big stream DMA issue (both HWDGE queues, in consumption
            # order: [win q0, wout q0, win q1, wout q1] x 4 rc x 2 nb) ----
            stream = {}
            qi = 0
            for q in range(NQ):
                for m in range(2):
                    for rc in range(nrc):
                        for nb in range(2):
                            w3t = sp.tile([128, CHUNK_R], BF16, tag="w3s")
                            eng = nc.sync if qi % 2 == 0 else nc.scalar
                            eng.dma_start(
                                w3t[:],
                                w3T_aps[m][q][nb * 128:(nb + 1) * 128,
                                              rc * CHUNK_R:(rc + 1) * CHUNK_R])
                            stream[(q, m, rc, nb)] = w3t
                            qi += 1

            # ---- parameter nets (tiny) ----------------------------------
            h0pre = pp.tile([128, 8], F32, tag="h0pre")
            nc.vector.tensor_scalar_mul(h0pre[:], w1_sb[:], t_bc[:, 0:1])
            nc.vector.tensor_add(h0pre[:], h0pre[:], b1_sb[:])
            h0_sb = pp.tile([128, 8], BF16, tag="h0")
            nc.scalar.activation(h0_sb[:], h0pre[:],
                                 mybir.ActivationFunctionType.Tanh)
            ps_h1 = ps_misc.tile([128, 8], F32, tag="misc")
            for k4 in range(4):
                for nb in range(2):
                    c = k4 * 2 + nb
                    for mb in range(2):
                        lhs = w2t_sb[:, k4 * 512 + mb * 256 + nb * 128:
                                     k4 * 512 + mb * 256 + nb * 128 + 128]
                        nc.tensor.matmul(ps_h1[:, c:c + 1], lhs,
                                         h0_sb[:, k4 * 2 + mb:k4 * 2 + mb + 1],
                                         start=(mb == 0), stop=(mb == 1))
            h1pre = pp.tile([128, 8], F32, tag="h1pre")
            h1_sb = pp.tile([128, 8], BF16, tag="h1")
            nc.vector.tensor_add(h1pre[:], ps_h1[:], b2_sb[:])
            nc.scalar.activation(h1_sb[:], h1pre[:],
                                 mybir.ActivationFunctionType.Tanh)

            # ---- heads: b and gate (local f, [128 f, 2] psum cols) ------
            b_loc = pp.tile([128, NQ], F32, tag="bloc")
            gate_loc = pp.tile([128, NQ], F32, tag="gateloc")
            gpre = pp.tile([128, NQ], F32, tag="gpre")
            for hb, dst, net in ((w3h_sb[0], b_loc, 2), (w3h_sb[1], gpre, 3)):
                ph = ps_misc.tile([128, NQ], F32, tag="misc", name="phd")
                for j in range(NQ):
                    for nb in range(2):
                        nc.tensor.matmul(
                            ph[:, j:j + 1],
                            hb[:, nb * FL + j * 128:nb * FL + (j + 1) * 128],
                            h1_sb[:, net * 2 + nb:net * 2 + nb + 1],
                            start=(nb == 0), stop=(nb == 1))
                bias = b3b_sb if net == 2 else b3gate_sb
                nc.vector.tensor_add(dst[:], ph[:], bias[:])
            nc.scalar.activation(gate_loc[:], gpre[:],
                                 mybir.ActivationFunctionType.Sigmoid)

            # ---- phase A: PE matvec + per-chunk AllGather ---------------
            cc_in = [dp.tile([128, CCW], BF16, tag=f"ccin{q}",
                             name=f"ccin{q}") for q in range(NQ)]
            cc_out = [dp.tile([n_cores, 128, CCW], BF16, tag=f"ccout{q}",
                              name=f"ccout{q}") for q in range(NQ)]
            ag_sb = []
            sg_f32 = pp.tile([128, NQ], F32, tag="sg")
            for q in range(NQ):
                mv = []
                for m, net in ((0, 0), (1, 1)):
                    pw = ps_mv.tile([128, FQ], F32, tag="mv", name=f"mv{m}")
                    for rc in range(nrc):
                        for j in range(cpr):
                            col = rc * cpr + j
                            for nb in range(2):
                                w3t = stream[(q, m, rc, nb)]
                                nc.tensor.matmul(
                                    pw[:, col:col + 1],
                                    w3t[:, j * 128:(j + 1) * 128],
                                    h1_sb[:, net * 2 + nb:net * 2 + nb + 1],
                                    start=(nb == 0), stop=(nb == 1))
                    mv.append(pw)
                cc_sb = cp.tile([128, CCW], BF16, tag="ccsb")
                # w_inT (+bias) -> cc cols [0, FQ)
                nc.vector.tensor_add(cc_sb[:, 0:FQ], mv[0][:],
                                     b3winT_sb[:, q * FQ:(q + 1) * FQ])
                woutTb = wp.tile([128, FQ], BF16, tag="woutTb")
                nc.vector.tensor_add(woutTb[:], mv[1][:],
                                     b3woutT_sb[:, q * FQ:(q + 1) * FQ])
                tpsB = ps_misc.tile([128, 128], BF16, tag="misc", name="tpsB")
                nc.tensor.transpose(tpsB[:], woutTb[:], eye_sb[:])
                # w_outg_fz = w_out_fz * gate -> cc cols [FQ, 2FQ)
                nc.vector.tensor_scalar_mul(cc_sb[:, FQ:2 * FQ], tpsB[:],
                                            gate_loc[:, q:q + 1])
                tpsA = ps_misc.tile([128, 128], BF16, tag="misc", name="tpsA")
                nc.tensor.transpose(tpsA[:], cc_sb[:, 0:FQ], eye_sb[:])
                prod = wp.tile([128, 128], F32, tag="sgprod")
                nc.vector.tensor_mul(prod[:], tpsA[:], cc_sb[:, FQ:2 * FQ])
                nc.vector.tensor_reduce(sg_f32[:, q:q + 1], prod[:],
                                        mybir.AxisListType.X, ADD)
                nc.vector.tensor_copy(cc_sb[:, 2 * FQ:2 * FQ + 1],
                                      sg_f32[:, q:q + 1])
                nc.vector.tensor_copy(cc_sb[:, 2 * FQ + 1:2 * FQ + 2],
                                      b_loc[:, q:q + 1])
                nc.gpsimd.dma_start(cc_in[q][:], cc_sb[:])
                nc.gpsimd.collective_compute(
                    "AllGather", mybir.AluOpType.bypass,
                    replica_groups=[list(range(n_cores))],
                    ins=[cc_in[q].opt()], outs=[cc_out[q].opt()])
                ag = pp.tile([128, n_cores * CCW], BF16, tag=f"ag{q}")
                nc.gpsimd.dma_start(
                    ag[:], cc_out[q].rearrange("k p c -> p k c"))
                ag_sb.append(ag)

            # ---- phase B: B-sharded batch matmuls over full F -----------
            # f-blocks ordered q-major so all of AG chunk 0 is consumed
            # before anything waits on AG chunk 1.
            nj = BL // BC
            nfb = F // 128
            b_f32 = pp.tile([128, nfb], F32, tag="bf32")
            pdz = [ps_dz.tile([128, BC], F32, tag=f"pdz{j}", name=f"pdz{j}")
                   for j in range(nj)]
            ptr = [ps_tr.tile([1, BC], F32, tag=f"ptr{j}", name=f"ptr{j}")
                   for j in range(nj)]
            ag3d = [ag_sb[q].rearrange("p (k c) -> p k c", c=CCW)
                    for q in range(NQ)]
            for q in range(NQ):
                nc.vector.tensor_copy(
                    b_f32[:, q * n_cores:(q + 1) * n_cores],
                    ag3d[q][:, :, 2 * FQ + 1])
                for kk in range(n_cores):
                    i = q * n_cores + kk
                    ag = ag_sb[q]
                    lhT = ag[:, kk * CCW:kk * CCW + FQ]
                    lhD = ag[:, kk * CCW + FQ:kk * CCW + 2 * FQ]
                    sgc = ag[:, kk * CCW + 2 * FQ:kk * CCW + 2 * FQ + 1]
                    for j in range(nj):
                        ph = ps_h.tile([128, BC], F32, tag="ph")
                        nc.tensor.matmul(ph[:], lhT,
                                         zt_sb[:, j * BC:(j + 1) * BC],
                                         start=True, stop=True)
                        h_bf = wp.tile([128, BC], BF16, tag="hbf")
                        nc.scalar.activation(h_bf[:], ph[:],
                                             mybir.ActivationFunctionType.Tanh,
                                             bias=b_f32[:, i:i + 1])
                        h2_bf = wp.tile([128, BC], BF16, tag="h2bf")
                        nc.vector.tensor_mul(h2_bf[:], h_bf[:], h_bf[:])
                        nc.tensor.matmul(pdz[j][:], lhD, h_bf[:],
                                         start=(i == 0), stop=(i == nfb - 1))
                        nc.tensor.matmul(ptr[j][:], sgc, h2_bf[:],
                                         start=(i == 0), stop=(i == nfb - 1))
            # trace constant: cneg = -sum_f sg / F (issued after all h2
            # muls so the DVE FIFO never blocks on the second AllGather)
            s1 = pp.tile([128, NQ], F32, tag="s1")
            for q in range(NQ):
                nc.vector.tensor_reduce(s1[:, q:q + 1],
                                        ag3d[q][:, :, 2 * FQ],
                                        mybir.AxisListType.X, ADD)
            s1t = pp.tile([128, 1], F32, tag="s1t")
            nc.vector.tensor_reduce(s1t[:], s1[:], mybir.AxisListType.X, ADD)
            s128 = pp.tile([128, 1], F32, tag="s128")
            from concourse import bass_isa
            nc.gpsimd.partition_all_reduce(s128[:], s1t[:], 128,
                                           bass_isa.ReduceOp.add)
            cneg = pp.tile([1, 1], F32, tag="cneg")
            nc.scalar.mul(cneg[:], s128[0:1, 0:1], -1.0 / F)
            for j in range(nj):
                dz_sb = wp.tile([128, BC], F32, tag="dzsb")
                nc.scalar.mul(dz_sb[:], pdz[j][:], 1.0 / F)
                nc.sync.dma_start(out_ap[0:Z, j * BC:(j + 1) * BC], dz_sb[:])
                tr_sb = wp.tile([1, BC], F32, tag="trsb")
                nc.scalar.activation(tr_sb[:], ptr[j][:],
                                     mybir.ActivationFunctionType.Identity,
                                     bias=cneg[0:1, 0:1], scale=1.0 / F)
                nc.sync.dma_start(out_ap[Z:Z + 1, j * BC:(j + 1) * BC],
                                  tr_sb[:])

    nc.compile()
    return nc


def host_prep(t, z_and_logpz, W1, B1, W2, B2, W3_win, b3_win,
              W3_wout, b3_wout, W3_b, b3_b, W3_gate, b3_gate,
              n_cores=N_CORES):
    """Shard + lay out the numpy inputs into per-core in_maps."""
    def col8(x):  # [4, 256] -> [128, 8] with col = k*2 + nb
        return np.ascontiguousarray(
            np.asarray(x, np.float32).reshape(4, 2, 128).transpose(2, 0, 1)
            .reshape(128, 8))

    t_in = np.asarray(t, np.float32).reshape(1, 1)
    w1c = col8(np.asarray(W1, np.float32)[:, :, 0])
    b1c = col8(B1)
    b2c = col8(B2)
    w2tc = np.ascontiguousarray(
        np.asarray(W2, np.float32).transpose(0, 2, 1)
        .reshape(4, 2, 128, 256).transpose(2, 0, 1, 3).reshape(128, 2048)
    ).astype(BF)
    w3win_bf = np.asarray(W3_win, np.float32).astype(BF)
    w3wout_bf = np.asarray(W3_wout, np.float32).astype(BF)
    w3b_bf = np.asarray(W3_b, np.float32).astype(BF)
    w3gate_bf = np.asarray(W3_gate, np.float32).astype(BF)
    b3win = np.asarray(b3_win, np.float32)
    b3wout = np.asarray(b3_wout, np.float32)
    z = np.asarray(z_and_logpz, np.float32)[:, :Z]
    ztb = np.ascontiguousarray(z.T).astype(BF)   # [Z, B]
    eye = np.eye(128, dtype=np.float32).astype(BF)

    rows = FL * Z            # per-core W3 rows (32768)
    in_maps = []
    for k in range(n_cores):
        r0 = k * rows
        f0 = k * FL
        im = {
            "t": t_in, "w1c": w1c, "b1c": b1c, "b2c": b2c, "w2tc": w2tc,
            "b3winT_c": np.ascontiguousarray(
                b3win[r0:r0 + rows].reshape(FL, Z).T),
            "b3woutT_c": np.ascontiguousarray(
                b3wout[r0:r0 + rows].reshape(FL, Z).T),
            "w3bT_c": np.ascontiguousarray(w3b_bf[f0:f0 + FL].T),
            "w3gateT_c": np.ascontiguousarray(w3gate_bf[f0:f0 + FL].T),
            "b3b_c": np.ascontiguousarray(
                np.asarray(b3_b, np.float32)[f0:f0 + FL].reshape(NQ, 128).T),
            "b3gate_c": np.ascontiguousarray(
                np.asarray(b3_gate, np.float32)[f0:f0 + FL].reshape(NQ, 128).T),
            "ztk": np.ascontiguousarray(ztb[:, k * BL:(k + 1) * BL]),
            "eyeb": eye,
        }
        for q in range(NQ):
            rq0 = r0 + q * RQ
            im[f"w3winT_c{q}"] = np.ascontiguousarray(
                w3win_bf[rq0:rq0 + RQ].T)
            im[f"w3woutT_c{q}"] = np.ascontiguousarray(
                w3wout_bf[rq0:rq0 + RQ].T)
        in_maps.append(im)
    return in_maps


_NC_CACHE = {}


def kernel(**inputs) -> np.ndarray:
    _ensure_ntff_hook()
    from concourse import bass_utils

    key = "full"
    if key not in _NC_CACHE:
        _NC_CACHE[key] = build_module()
    nc = _NC_CACHE[key]

    in_maps = host_prep(**inputs)
    res = bass_utils.run_bass_kernel_spmd(nc, in_maps, list(range(N_CORES)))
    out = np.empty((B, Z + 1), np.float32)
    for k in range(N_CORES):
        out[k * BL:(k + 1) * BL, :] = res.results[k]["out"].T
    return out


# revision 18
# speedup vs baseline: 2.0509x; 1.1463x over previous
"""Trainium2 Bass kernel for nn_ContinousNormalizingFlowRHS.

Computes, for z in R^{B x Z} and scalar time t:
  h0 = tanh(W1*t + B1); h1 = tanh(einsum('knm,km->kn', W2, h0) + B2)
  w_in  = (W3_win  @ h1[0] + b3_win ).reshape(F, Z)
  w_out = (W3_wout @ h1[1] + b3_wout).reshape(F, Z)
  b     =  W3_b    @ h1[2] + b3_b
  gate  = sigmoid(W3_gate @ h1[3] + b3_gate)
  h = tanh(z @ w_in.T + b); dz = (h*gate) @ w_out / F
  trace = ((1-h^2)*gate) @ (sum(w_in*w_out,1)) / F
  out = concat([dz, -trace[:,None]], -1)

Strategy (8 NeuronCores, single SPMD launch):
  Phase A (F-sharded): each core streams its 1/8 of W3_win/W3_wout
  (bf16, [N, rows] transposed layout) on the two HWDGE queues and runs
  the matvec entirely on the PE as a stream of FWL stationary loads
  with a 1-column moving h1.  The psum naturally lands in w_inT
  ([z, f]) layout.  w_out is transposed on-chip, gate/bias folded, and
  the per-f trace weights sg are reduced locally.
  Handoff: two chunked AllGathers (one per local f-half) move the tiny
  per-core (w_inT, w_outg_fz, sg, b) slices to every core, overlapped
  with the second half of the stream.
  Phase B (B-sharded): each core computes only its own B/8 batch rows
  against the full F, accumulating dz/trace in PSUM across all 16
  f-blocks, then writes its output shard directly (no ReduceScatter).
"""

import sys
import types
import numpy as np
import ml_dtypes

BF = ml_dtypes.bfloat16

# problem sizes (hardcoded per contract)
Z = 128
N = 256
F = 2048
B = 8192
N_CORES = 8

FL = F // N_CORES          # local f per core (256)
NQ = 2                     # AllGather chunks (f-halves of 128)
FQ = FL // NQ              # f per chunk (128)
RQ = FQ * Z                # W3 rows per chunk (16384)
CHUNK_R = 2048             # rows per streamed tile ([128, 2048] bf16)
BL = B // N_CORES          # batch shard (1024)
BC = 512                   # batch columns per stage-B psum chunk
CCW = 2 * FQ + 2           # cc payload cols: w_inT | w_outg_fz | sg | b


def _ensure_ntff_hook():
    """run_bass_kernel_spmd(trace=True) under axon needs antenv.axon_hooks."""
    if 'antenv.axon_hooks' in sys.modules:
        return
    try:
        from trn_agent_boot.trn_boot import _ntff_profile_via_ctypes
        hook = _ntff_profile_via_ctypes('/opt/axon/libaxon_pjrt.so')
    except Exception:
        hook = None
    try:
        import antenv
    except Exception:
        return
    mod = types.ModuleType('antenv.axon_hooks')
    mod.get_axon_ntff_profile_hook = lambda: hook
    mod.set_axon_ntff_profile_hook = lambda h: None
    sys.modules['antenv.axon_hooks'] = mod
    antenv.axon_hooks = mod


def build_module(n_cores=N_CORES, debug=False):
    import concourse.tile as tile
    from concourse import bacc, mybir

    F32 = mybir.dt.float32
    BF16 = mybir.dt.bfloat16
    ADD = mybir.AluOpType.add

    nrc = RQ // CHUNK_R          # streamed tiles per (chunk, matrix, nb) (4)
    cpr = CHUNK_R // Z           # psum cols per streamed tile (32)

    nc = bacc.Bacc("TRN2", target_bir_lowering=False, debug=debug,
                   num_devices=n_cores)

    def inp(name, shape, dt):
        return nc.dram_tensor(name, shape, dt, kind="ExternalInput").ap()

    t_ap = inp("t", [1, 1], F32)
    w1_ap = inp("w1c", [128, 8], F32)
    b1_ap = inp("b1c", [128, 8], F32)
    b2_ap = inp("b2c", [128, 8], F32)
    w2t_ap = inp("w2tc", [128, 2048], BF16)
    w3T_aps = [[inp(f"w3{m}T_c{q}", [N, RQ], BF16) for q in range(NQ)]
               for m in ("win", "wout")]
    b3winT_ap = inp("b3winT_c", [128, FL], F32)
    b3woutT_ap = inp("b3woutT_c", [128, FL], F32)
    w3bT_ap = inp("w3bT_c", [N, FL], BF16)
    w3gateT_ap = inp("w3gateT_c", [N, FL], BF16)
    b3b_ap = inp("b3b_c", [128, NQ], F32)
    b3gate_ap = inp("b3gate_c", [128, NQ], F32)
    zt_ap = inp("ztk", [128, BL], BF16)
    eye_ap = inp("eyeb", [128, 128], BF16)
    out_ap = nc.dram_tensor("out", [Z + 1, BL], F32,
                            kind="ExternalOutput").ap()

    with tile.TileContext(nc) as tc:
        with tc.tile_pool(name="persist", bufs=1) as pp, \
             tc.tile_pool(name="strm_sy", bufs=8) as sp_sy, \
             tc.tile_pool(name="strm_sc", bufs=8) as sp_sc, \
             tc.tile_pool(name="strm_gp", bufs=4) as sp_gp, \
             tc.tile_pool(name="work", bufs=3) as wp, \
             tc.tile_pool(name="ccsb", bufs=2) as cp, \
             tc.tile_pool(name="ps_misc", bufs=1, space="PSUM") as ps_misc, \
             tc.tile_pool(name="ps_mv", bufs=1, space="PSUM") as ps_mv, \
             tc.tile_pool(name="ps_h", bufs=2, space="PSUM") as ps_h, \
             tc.tile_pool(name="ps_dz", bufs=1, space="PSUM") as ps_dz, \
             tc.tile_pool(name="ps_tr", bufs=1, space="PSUM") as ps_tr, \
             tc.tile_pool(name="dram", bufs=1, space="DRAM") as dp:

            # ---- small loads (SWDGE queue; issue before the big stream) --
            t_bc = pp.tile([128, 1], F32, tag="tbc")
            nc.gpsimd.dma_start(t_bc[:], t_ap.broadcast_to([128, 1]))
            w1_sb = pp.tile([128, 8], F32, tag="w1")
            b1_sb = pp.tile([128, 8], F32, tag="b1")
            b2_sb = pp.tile([128, 8], F32, tag="b2")
            w2t_sb = pp.tile([128, 2048], BF16, tag="w2t")
            nc.gpsimd.dma_start(w1_sb[:], w1_ap[:])
            nc.gpsimd.dma_start(b1_sb[:], b1_ap[:])
            nc.gpsimd.dma_start(b2_sb[:], b2_ap[:])
            nc.gpsimd.dma_start(w2t_sb[:], w2t_ap[:])
            zt_sb = pp.tile([128, BL], BF16, tag="zt")
            nc.gpsimd.dma_start(zt_sb[:], zt_ap[:])
            eye_sb = pp.tile([128, 128], BF16, tag="eye")
            nc.gpsimd.dma_start(eye_sb[:], eye_ap[:])
            b3winT_sb = pp.tile([128, FL], F32, tag="b3winT")
            b3woutT_sb = pp.tile([128, FL], F32, tag="b3woutT")
            nc.gpsimd.dma_start(b3winT_sb[:], b3winT_ap[:])
            nc.gpsimd.dma_start(b3woutT_sb[:], b3woutT_ap[:])
            w3h_sb = []
            for m, ap in (("b", w3bT_ap), ("gate", w3gateT_ap)):
                hb = pp.tile([128, 2 * FL], BF16, tag=f"w3{m}T")
                nc.gpsimd.dma_start(
                    hb[:], ap.rearrange("(nb p) fl -> p nb fl", p=128))
                w3h_sb.append(hb)
            b3b_sb = pp.tile([128, NQ], F32, tag="b3b")
            b3gate_sb = pp.tile([128, NQ], F32, tag="b3gate")
            nc.gpsimd.dma_start(b3b_sb[:], b3b_ap[:])
            nc.gpsimd.dma_start(b3gate_sb[:], b3gate_ap[:])

            # ---- parameter nets (tiny) ----------------------------------
            h0pre = pp.tile([128, 8], F32, tag="h0pre")
            nc.vector.tensor_scalar_mul(h0pre[:], w1_sb[:], t_bc[:, 0:1])
            nc.vector.tensor_add(h0pre[:], h0pre[:], b1_sb[:])
            h0_sb = pp.tile([128, 8], BF16, tag="h0")
            nc.scalar.activation(h0_sb[:], h0pre[:],
                                 mybir.ActivationFunctionType.Tanh)
            ps_h1 = ps_misc.tile([128, 8], F32, tag="misc")
            for k4 in range(4):
                for nb in range(2):
                    c = k4 * 2 + nb
                    for mb in range(2):
                        lhs = w2t_sb[:, k4 * 512 + mb * 256 + nb * 128:
                                     k4 * 512 + mb * 256 + nb * 128 + 128]
                        nc.tensor.matmul(ps_h1[:, c:c + 1], lhs,
                                         h0_sb[:, k4 * 2 + mb:k4 * 2 + mb + 1],
                                         start=(mb == 0), stop=(mb == 1))
            h1pre = pp.tile([128, 8], F32, tag="h1pre")
            h1_sb = pp.tile([128, 8], BF16, tag="h1")
            nc.vector.tensor_add(h1pre[:], ps_h1[:], b2_sb[:])
            nc.scalar.activation(h1_sb[:], h1pre[:],
                                 mybir.ActivationFunctionType.Tanh)

            # ---- heads: b and gate (local f, [128 f, 2] psum cols) ------
            b_loc = pp.tile([128, NQ], F32, tag="bloc")
            gate_loc = pp.tile([128, NQ], F32, tag="gateloc")
            gpre = pp.tile([128, NQ], F32, tag="gpre")
            for hb, dst, net in ((w3h_sb[0], b_loc, 2), (w3h_sb[1], gpre, 3)):
                ph = ps_misc.tile([128, NQ], F32, tag="misc", name="phd")
                for j in range(NQ):
                    for nb in range(2):
                        nc.tensor.matmul(
                            ph[:, j:j + 1],
                            hb[:, nb * FL + j * 128:nb * FL + (j + 1) * 128],
                            h1_sb[:, net * 2 + nb:net * 2 + nb + 1],
                            start=(nb == 0), stop=(nb == 1))
                bias = b3b_sb if net == 2 else b3gate_sb
                nc.vector.tensor_add(dst[:], ph[:], bias[:])
            nc.scalar.activation(gate_loc[:], gpre[:],
                                 mybir.ActivationFunctionType.Sigmoid)

            # ---- phase A: PE matvec + per-chunk AllGather ---------------
            cc_in = [dp.tile([128, CCW], BF16, tag=f"ccin{q}",
                             name=f"ccin{q}") for q in range(NQ)]
            cc_out = [dp.tile([n_cores, 128, CCW], BF16, tag=f"ccout{q}",
                              name=f"ccout{q}") for q in range(NQ)]
            ag_sb = []
            sg_f32 = pp.tile([128, NQ], F32, tag="sg")
            # engine stripe: 2x sync, 2x scalar, 1x gpsimd per 5 tiles —
            # ring pacing on each queue then tracks consumption order.
            stripe = [(nc.sync, sp_sy), (nc.scalar, sp_sc),
                      (nc.sync, sp_sy), (nc.scalar, sp_sc),
                      (nc.gpsimd, sp_gp)]
            for q in range(NQ):
                # chunk's stream DMA issue, in consumption order
                stream = {}
                for m in range(2):
                    for rc in range(nrc):
                        for nb in range(2):
                            ti = m * 2 * nrc + rc * 2 + nb
                            eng, pool = stripe[ti % 5]
                            w3t = pool.tile([128, CHUNK_R], BF16, tag="w3s")
                            eng.dma_start(
                                w3t[:],
                                w3T_aps[m][q][nb * 128:(nb + 1) * 128,
                                              rc * CHUNK_R:(rc + 1) * CHUNK_R])
                            stream[(m, rc, nb)] = w3t
                mv = []
                for m, net in ((0, 0), (1, 1)):
                    pw = ps_mv.tile([128, FQ], F32, tag="mv", name=f"mv{m}")
                    for rc in range(nrc):
                        for j in range(cpr):
                            col = rc * cpr + j
                            for nb in range(2):
                                w3t = stream[(m, rc, nb)]
                                nc.tensor.matmul(
                                    pw[:, col:col + 1],
                                    w3t[:, j * 128:(j + 1) * 128],
                                    h1_sb[:, net * 2 + nb:net * 2 + nb + 1],
                                    start=(nb == 0), stop=(nb == 1))
                    mv.append(pw)
                cc_sb = cp.tile([128, CCW], BF16, tag="ccsb")
                # w_inT (+bias) -> cc cols [0, FQ)
                nc.vector.tensor_add(cc_sb[:, 0:FQ], mv[0][:],
                                     b3winT_sb[:, q * FQ:(q + 1) * FQ])
                woutTb = wp.tile([128, FQ], BF16, tag="woutTb")
                nc.vector.tensor_add(woutTb[:], mv[1][:],
                                     b3woutT_sb[:, q * FQ:(q + 1) * FQ])
                tpsB = ps_misc.tile([128, 128], BF16, tag="misc", name="tpsB")
                nc.tensor.transpose(tpsB[:], woutTb[:], eye_sb[:])
                # w_outg_fz = w_out_fz * gate -> cc cols [FQ, 2FQ)
                nc.vector.tensor_scalar_mul(cc_sb[:, FQ:2 * FQ], tpsB[:],
                                            gate_loc[:, q:q + 1])
                tpsA = ps_misc.tile([128, 128], BF16, tag="misc", name="tpsA")
                nc.tensor.transpose(tpsA[:], cc_sb[:, 0:FQ], eye_sb[:])
                prod = wp.tile([128, 128], F32, tag="sgprod")
                nc.vector.tensor_mul(prod[:], tpsA[:], cc_sb[:, FQ:2 * FQ])
                nc.vector.tensor_reduce(sg_f32[:, q:q + 1], prod[:],
                                        mybir.AxisListType.X, ADD)
                nc.vector.tensor_copy(cc_sb[:, 2 * FQ:2 * FQ + 1],
                                      sg_f32[:, q:q + 1])
                nc.vector.tensor_copy(cc_sb[:, 2 * FQ + 1:2 * FQ + 2],
                                      b_loc[:, q:q + 1])
                nc.gpsimd.dma_start(cc_in[q][:], cc_sb[:])
                nc.gpsimd.collective_compute(
                    "AllGather", mybir.AluOpType.bypass,
                    replica_groups=[list(range(n_cores))],
                    ins=[cc_in[q].opt()], outs=[cc_out[q].opt()])
                ag = pp.tile([128, n_cores * CCW], BF16, tag=f"ag{q}")
                nc.gpsimd.dma_start(
                    ag[:], cc_out[q].rearrange("k p c -> p k c"))
                ag_sb.append(ag)

            # ---- phase B: B-sharded batch matmuls over full F -----------
            # f-blocks ordered q-major so all of AG chunk 0 is consumed
            # before anything waits on AG chunk 1.
            nj = BL // BC
            nfb = F // 128
            b_f32 = pp.tile([128, nfb], F32, tag="bf32")
            pdz = [ps_dz.tile([128, BC], F32, tag=f"pdz{j}", name=f"pdz{j}")
                   for j in range(nj)]
            ptr = [ps_tr.tile([1, BC], F32, tag=f"ptr{j}", name=f"ptr{j}")
                   for j in range(nj)]
            ag3d = [ag_sb[q].rearrange("p (k c) -> p k c", c=CCW)
                    for q in range(NQ)]
            for q in range(NQ):
                nc.vector.tensor_copy(
                    b_f32[:, q * n_cores:(q + 1) * n_cores],
                    ag3d[q][:, :, 2 * FQ + 1])
                for kk in range(n_cores):
                    i = q * n_cores + kk
                    ag = ag_sb[q]
                    lhT = ag[:, kk * CCW:kk * CCW + FQ]
                    lhD = ag[:, kk * CCW + FQ:kk * CCW + 2 * FQ]
                    sgc = ag[:, kk * CCW + 2 * FQ:kk * CCW + 2 * FQ + 1]
                    for j in range(nj):
                        ph = ps_h.tile([128, BC], F32, tag="ph")
                        nc.tensor.matmul(ph[:], lhT,
                                         zt_sb[:, j * BC:(j + 1) * BC],
                                         start=True, stop=True)
                        h_bf = wp.tile([128, BC], BF16, tag="hbf")
                        nc.scalar.activation(h_bf[:], ph[:],
                                             mybir.ActivationFunctionType.Tanh,
                                             bias=b_f32[:, i:i + 1])
                        h2_bf = wp.tile([128, BC], BF16, tag="h2bf")
                        nc.vector.tensor_mul(h2_bf[:], h_bf[:], h_bf[:])
                        nc.tensor.matmul(pdz[j][:], lhD, h_bf[:],
                                         start=(i == 0), stop=(i == nfb - 1))
                        nc.tensor.matmul(ptr[j][:], sgc, h2_bf[:],
                                         start=(i == 0), stop=(i == nfb - 1))
            # trace constant: cneg = -sum_f sg / F (issued after all h2
            # muls so the DVE FIFO never blocks on the second AllGather)
            s1 = pp.tile([128, NQ], F32, tag="s1")
            for q in range(NQ):
                nc.vector.tensor_reduce(s1[:, q:q + 1],
                                        ag3d[q][:, :, 2 * FQ],
                                        mybir.AxisListType.X, ADD)
            s1t = pp.tile([128, 1], F32, tag="s1t")
            nc.vector.tensor_reduce(s1t[:], s1[:], mybir.AxisListType.X, ADD)
            s128 = pp.tile([128, 1], F32, tag="s128")
            from concourse import bass_isa
            nc.gpsimd.partition_all_reduce(s128[:], s1t[:], 128,
                                           bass_isa.ReduceOp.add)
            cneg = pp.tile([1, 1], F32, tag="cneg")
            nc.scalar.mul(cneg[:], s128[0:1, 0:1], -1.0 / F)
            for j in range(nj):
                dz_sb = wp.tile([128, BC], F32, tag="dzsb")
                nc.scalar.mul(dz_sb[:], pdz[j][:], 1.0 / F)
                nc.sync.dma_start(out_ap[0:Z, j * BC:(j + 1) * BC], dz_sb[:])
                tr_sb = wp.tile([1, BC], F32, tag="trsb")
                nc.scalar.activation(tr_sb[:], ptr[j][:],
                                     mybir.ActivationFunctionType.Identity,
                                     bias=cneg[0:1, 0:1], scale=1.0 / F)
                nc.sync.dma_start(out_ap[Z:Z + 1, j * BC:(j + 1) * BC],
                                  tr_sb[:])

    nc.compile()
    return nc


def host_prep(t, z_and_logpz, W1, B1, W2, B2, W3_win, b3_win,
              W3_wout, b3_wout, W3_b, b3_b, W3_gate, b3_gate,
              n_cores=N_CORES):
    """Shard + lay out the numpy inputs into per-core in_maps."""
    def col8(x):  # [4, 256] -> [128, 8] with col = k*2 + nb
        return np.ascontiguousarray(
            np.asarray(x, np.float32).reshape(4, 2, 128).transpose(2, 0, 1)
            .reshape(128, 8))

    t_in = np.asarray(t, np.float32).reshape(1, 1)
    w1c = col8(np.asarray(W1, np.float32)[:, :, 0])
    b1c = col8(B1)
    b2c = col8(B2)
    w2tc = np.ascontiguousarray(
        np.asarray(W2, np.float32).transpose(0, 2, 1)
        .reshape(4, 2, 128, 256).transpose(2, 0, 1, 3).reshape(128, 2048)
    ).astype(BF)
    w3win_bf = np.asarray(W3_win, np.float32).astype(BF)
    w3wout_bf = np.asarray(W3_wout, np.float32).astype(BF)
    w3b_bf = np.asarray(W3_b, np.float32).astype(BF)
    w3gate_bf = np.asarray(W3_gate, np.float32).astype(BF)
    b3win = np.asarray(b3_win, np.float32)
    b3wout = np.asarray(b3_wout, np.float32)
    z = np.asarray(z_and_logpz, np.float32)[:, :Z]
    ztb = np.ascontiguousarray(z.T).astype(BF)   # [Z, B]
    eye = np.eye(128, dtype=np.float32).astype(BF)

    rows = FL * Z            # per-core W3 rows (32768)
    in_maps = []
    for k in range(n_cores):
        r0 = k * rows
        f0 = k * FL
        im = {
            "t": t_in, "w1c": w1c, "b1c": b1c, "b2c": b2c, "w2tc": w2tc,
            "b3winT_c": np.ascontiguousarray(
                b3win[r0:r0 + rows].reshape(FL, Z).T),
            "b3woutT_c": np.ascontiguousarray(
                b3wout[r0:r0 + rows].reshape(FL, Z).T),
            "w3bT_c": np.ascontiguousarray(w3b_bf[f0:f0 + FL].T),
            "w3gateT_c": np.ascontiguousarray(w3gate_bf[f0:f0 + FL].T),
            "b3b_c": np.ascontiguousarray(
                np.asarray(b3_b, np.float32)[f0:f0 + FL].reshape(NQ, 128).T),
            "b3gate_c": np.ascontiguousarray(
                np.asarray(b3_gate, np.float32)[f0:f0 + FL].reshape(NQ, 128).T),
            "ztk": np.ascontiguousarray(ztb[:, k * BL:(k + 1) * BL]),
            "eyeb": eye,
        }
        for q in range(NQ):
            rq0 = r0 + q * RQ
            im[f"w3winT_c{q}"] = np.ascontiguousarray(
                w3win_bf[rq0:rq0 + RQ].T)
            im[f"w3woutT_c{q}"] = np.ascontiguousarray(
                w3wout_bf[rq0:rq0 + RQ].T)
        in_maps.append(im)
    return in_maps


_NC_CACHE = {}


def kernel(**inputs) -> np.ndarray:
    _ensure_ntff_hook()
    from concourse import bass_utils

    key = "full"
    if key not in _NC_CACHE:
        _NC_CACHE[key] = build_module()
    nc = _NC_CACHE[key]

    in_maps = host_prep(**inputs)
    res = bass_utils.run_bass_kernel_spmd(nc, in_maps, list(range(N_CORES)))
    out = np.empty((B, Z + 1), np.float32)
    for k in range(N_CORES):
        out[k * BL:(k + 1) * BL, :] = res.results[k]["out"].T
    return out
